# revision 4
# baseline (speedup 1.0000x reference)
"""MemoryGate kernel for Trainium2 (8 NeuronCores, SPMD).

Math (per batch b):
    mp   = memory[b] @ W_mem.T                      [M, D]
    S    = hidden[b] @ mp.T / sqrt(D)               [N, M]
    A    = softmax(S, axis=-1)
    ctx  = A @ mp                                   [N, D]
    gate = sigmoid(hidden @ Wg_h.T + ctx @ Wg_c.T + b_gate)
    out  = rmsnorm(hidden + gate * ctx) * norm_w

Sharding: 8 cores = 4 batches x 2 N-halves. Each core computes mp for its
batch (duplicated across the pair) and processes N/2 = 2048 rows.

All matmul operands are bf16 (PE full rate, FWL weight loads, half the
DMA/SBUF traffic of fp32). PSUM accumulates fp32; softmax stats, gate
output, the fused residual path and rmsnorm stay fp32.

Layout strategy (per core, all [partition, free]):
    hiddenT [D, BN]  (host pre-transposed)  -> lhsT for scores and gate-G1
    memT    [E, M],  WmT [E, D]             -> stage A operands
    mpT     [D, M] (scaled by 1/sqrt(D)), mp [M, D] -> DRAM scratch
    scores computed as [n-part, m-free]; softmax along free dim (no max
    subtraction needed: scores ~ N(0,1), exp is safe in fp32)
    attn transposed on PE (128x128 blocks) -> attnT lhsT for ctx
    ctx transposed on PE -> ctxT lhsT for gate-G2
    b_gate added via a K=1 matmul into the same PSUM accumulation
    rmsnorm along free dim in [n-part, d-free] layout
"""

import math
import os
import sys

for _p in ("/opt/trn_rl_repo", "/root/.axon_site/_ro/trn_rl_repo"):
    if os.path.isdir(_p) and _p not in sys.path:
        sys.path.append(_p)

import numpy as np

P = 128


def build_program(BN=2048, M=2048, D=2048, E=1024, NB=512, FC=512,
                  stop_after=None, b_repeat=1):
    """Build the per-core Bass program. All shapes must divide evenly.

    stop_after: debug aid — truncate the program after a named phase
    ("A", "scores", "attnT", "ctx", "ctxT", "gate"); None = full kernel.
    """
    import concourse.tile as tile
    from concourse import bacc, mybir

    f32 = mybir.dt.float32
    bf16 = mybir.dt.bfloat16
    AF = mybir.ActivationFunctionType
    ALU = mybir.AluOpType
    AX = mybir.AxisListType

    kE, kD, mT, nT = E // P, D // P, M // P, NB // P
    NBLK = BN // NB
    mFC, dFC = M // FC, D // FC
    SCALE = 1.0 / math.sqrt(D)
    EPS = 1e-6

    nc = bacc.Bacc("TRN2", target_bir_lowering=False, debug=False)

    hT = nc.dram_tensor("hiddenT", [D, BN], bf16, kind="ExternalInput")
    hid = nc.dram_tensor("hidden", [BN, D], f32, kind="ExternalInput")
    memT = nc.dram_tensor("memT", [E, M], bf16, kind="ExternalInput")
    WmT = nc.dram_tensor("WmT", [E, D], bf16, kind="ExternalInput")
    WghT = nc.dram_tensor("WghT", [D, D], bf16, kind="ExternalInput")
    WgcT = nc.dram_tensor("WgcT", [D, D], bf16, kind="ExternalInput")
    bg = nc.dram_tensor("b_gate", [1, D], bf16, kind="ExternalInput")
    nw = nc.dram_tensor("norm_w", [1, D], f32, kind="ExternalInput")
    idd = nc.dram_tensor("ident", [P, P], bf16, kind="ExternalInput")
    oned = nc.dram_tensor("ones", [1, P], bf16, kind="ExternalInput")
    out = nc.dram_tensor("out", [BN, D], f32, kind="ExternalOutput")

    with tile.TileContext(nc) as tc:
        with (
            tc.tile_pool(name="dram", bufs=1, space="DRAM") as dpool,
            tc.tile_pool(name="const", bufs=1) as const,
        ):
            mp_d = dpool.tile([M, D], bf16, tag="mp", name="mp_d")

            ident = const.tile([P, P], bf16, tag="ident", name="ident_sb")
            nc.sync.dma_start(ident, idd[:])
            ones_sb = const.tile([1, P], bf16, tag="ones", name="ones_sb")
            nc.sync.dma_start(ones_sb, oned[:])
            bias_sb = const.tile([1, D], bf16, tag="bias", name="bias_sb")
            nc.sync.dma_start(bias_sb, bg[:])
            nw_sb = const.tile([P, D], f32, tag="nw", name="nw_sb")
            nc.gpsimd.dma_start(nw_sb, nw[:].partition_broadcast(P))
            eps_t = const.tile([P, 1], f32, tag="eps", name="eps_sb")
            nc.vector.memset(eps_t, EPS)

            # mpT stays resident in SBUF for the whole kernel (16 MiB bf16)
            hold_cm = tc.tile_pool(name="hold", bufs=1)
            hold = hold_cm.__enter__()
            mpT_sb = hold.tile([P, kD, M], bf16, tag="mpT", name="mpT_sb")

            # ---------------- Stage A: mpT (scaled, SBUF) and mp -> DRAM ----
            with (
                tc.tile_pool(name="a_in", bufs=1) as a_in,
                tc.tile_pool(name="a_st", bufs=4) as a_st,
                tc.tile_pool(name="a_ps", bufs=4, space="PSUM") as a_ps,
            ):
                memT_sb = a_in.tile([P, kE, M], bf16, tag="memT", name="memT_sb")
                WmT_sb = a_in.tile([P, kE, D], bf16, tag="WmT", name="WmT_sb")
                for k in range(kE):
                    nc.sync.dma_start(memT_sb[:, k, :], memT[k * P:(k + 1) * P, :])
                    nc.sync.dma_start(WmT_sb[:, k, :], WmT[k * P:(k + 1) * P, :])
                # A1: mpT[d, m] = sum_e WmT[e, d] * memT[e, m], scaled
                for dp in range(kD):
                    for mc in range(mFC):
                        ps = a_ps.tile([P, FC], f32, tag="ps", name=f"a1ps{dp}_{mc}")
                        for k in range(kE):
                            nc.tensor.matmul(
                                ps,
                                WmT_sb[:, k, dp * P:(dp + 1) * P],
                                memT_sb[:, k, mc * FC:(mc + 1) * FC],
                                start=(k == 0), stop=(k == kE - 1),
                            )
                        nc.scalar.mul(
                            mpT_sb[:, dp, mc * FC:(mc + 1) * FC], ps, SCALE)
                # A2: mp[m, d] = sum_e memT[e, m] * WmT[e, d]
                for mp_ in range(mT):
                    for dc in range(dFC):
                        ps = a_ps.tile([P, FC], f32, tag="ps", name=f"a2ps{mp_}_{dc}")
                        for k in range(kE):
                            nc.tensor.matmul(
                                ps,
                                memT_sb[:, k, mp_ * P:(mp_ + 1) * P],
                                WmT_sb[:, k, dc * FC:(dc + 1) * FC],
                                start=(k == 0), stop=(k == kE - 1),
                            )
                        st = a_st.tile([P, FC], bf16, tag="st", name=f"a2st{mp_}_{dc}")
                        nc.scalar.copy(st, ps)
                        nc.sync.dma_start(
                            mp_d[mp_ * P:(mp_ + 1) * P, dc * FC:(dc + 1) * FC], st)

            # ---------------- Stage B: per N-block pipeline -----------------
            with (
                tc.tile_pool(name="b_big", bufs=1) as bb,
                tc.tile_pool(name="b_strm", bufs=6) as strm,
                tc.tile_pool(name="b_sm", bufs=2) as sm,
                tc.tile_pool(name="b_ps", bufs=6, space="PSUM") as bps,
            ):
                for rep_blk in range(b_repeat * NBLK):
                    blk = rep_blk % NBLK
                    n0 = blk * NB
                    hT_sb = bb.tile([P, kD, NB], bf16, tag="hT", name=f"hT{rep_blk}")
                    for k in range(kD):
                        nc.sync.dma_start(hT_sb[:, k, :], hT[k * P:(k + 1) * P, n0:n0 + NB])

                    if stop_after == "A":
                        continue
                    # scores + exp (+ row-chunk sums)
                    attn = bb.tile([P, nT, M], bf16, tag="attn", name=f"attn{rep_blk}")
                    sums = sm.tile([P, nT * mFC], f32, tag="sums", name=f"sums{rep_blk}")
                    rs = sm.tile([P, nT], f32, tag="rs", name=f"rs{rep_blk}")
                    for mc in range(mFC):
                        pss = [bps.tile([P, FC], f32, tag="ps", name=f"sc{rep_blk}_{mc}_{i}")
                               for i in range(nT)]
                        for k in range(kD):
                            for i in range(nT):
                                nc.tensor.matmul(
                                    pss[i], hT_sb[:, k, i * P:(i + 1) * P],
                                    mpT_sb[:, k, mc * FC:(mc + 1) * FC],
                                    start=(k == 0), stop=(k == kD - 1))
                        for i in range(nT):
                            nc.scalar.activation(
                                attn[:, i, mc * FC:(mc + 1) * FC], pss[i], AF.Exp,
                                accum_out=sums[:, i * mFC + mc: i * mFC + mc + 1])
                    # softmax denominators; normalize attn in place
                    for i in range(nT):
                        nc.vector.reduce_sum(
                            out=rs[:, i:i + 1], in_=sums[:, i * mFC:(i + 1) * mFC], axis=AX.X)
                    nc.vector.reciprocal(rs, rs)
                    for i in range(nT):
                        nc.scalar.mul(attn[:, i, :], attn[:, i, :], rs[:, i:i + 1])

                    if stop_after == "scores":
                        continue
                    # transpose attn -> attnT
                    attnT = bb.tile([P, mT, NB], bf16, tag="attnT", name=f"attnT{rep_blk}")
                    for mt in range(mT):
                        tp = bps.tile([P, NB], bf16, tag="ps", name=f"tpa{rep_blk}_{mt}")
                        for i in range(nT):
                            nc.tensor.transpose(
                                tp[:, i * P:(i + 1) * P], attn[:, i, mt * P:(mt + 1) * P], ident)
                        nc.vector.tensor_copy(attnT[:, mt, :], tp)

                    if stop_after == "attnT":
                        continue
                    # ctx = attn @ mp
                    ctxt = bb.tile([P, nT, D], bf16, tag="ctx", name=f"ctx{rep_blk}")
                    for dc in range(dFC):
                        pss = [bps.tile([P, FC], f32, tag="ps", name=f"cx{rep_blk}_{dc}_{i}")
                               for i in range(nT)]
                        for mt in range(mT):
                            ch = strm.tile([P, FC], bf16, tag="rhs", name=f"c_ch{rep_blk}_{dc}_{mt}")
                            nc.sync.dma_start(ch, mp_d[mt * P:(mt + 1) * P, dc * FC:(dc + 1) * FC])
                            for i in range(nT):
                                nc.tensor.matmul(
                                    pss[i], attnT[:, mt, i * P:(i + 1) * P], ch,
                                    start=(mt == 0), stop=(mt == mT - 1))
                        for i in range(nT):
                            nc.scalar.copy(ctxt[:, i, dc * FC:(dc + 1) * FC], pss[i])

                    if stop_after == "ctx":
                        continue
                    # transpose ctx -> ctxT (reuses attnT's slot)
                    ctxT = bb.tile([P, kD, NB], bf16, tag="attnT", name=f"ctxT{rep_blk}")
                    for dt_ in range(kD):
                        tp = bps.tile([P, NB], bf16, tag="ps", name=f"tpc{rep_blk}_{dt_}")
                        for i in range(nT):
                            nc.tensor.transpose(
                                tp[:, i * P:(i + 1) * P], ctxt[:, i, dt_ * P:(dt_ + 1) * P], ident)
                        nc.vector.tensor_copy(ctxT[:, dt_, :], tp)

                    if stop_after == "ctxT":
                        continue
                    # gate = sigmoid(hidden @ WghT + ctx @ WgcT + b_gate)
                    gate = bb.tile([P, nT, D], bf16, tag="attn", name=f"gate{rep_blk}")
                    for dc in range(dFC):
                        pss = [bps.tile([P, FC], f32, tag="ps", name=f"gt{rep_blk}_{dc}_{i}")
                               for i in range(nT)]
                        for k in range(kD):
                            ch = strm.tile([P, FC], bf16, tag="rhs", name=f"g1ch{rep_blk}_{dc}_{k}")
                            nc.sync.dma_start(ch, WghT[k * P:(k + 1) * P, dc * FC:(dc + 1) * FC])
                            for i in range(nT):
                                nc.tensor.matmul(
                                    pss[i], hT_sb[:, k, i * P:(i + 1) * P], ch,
                                    start=(k == 0), stop=False)
                        for k in range(kD):
                            ch = strm.tile([P, FC], bf16, tag="rhs", name=f"g2ch{rep_blk}_{dc}_{k}")
                            nc.sync.dma_start(ch, WgcT[k * P:(k + 1) * P, dc * FC:(dc + 1) * FC])
                            for i in range(nT):
                                nc.tensor.matmul(
                                    pss[i], ctxT[:, k, i * P:(i + 1) * P], ch,
                                    start=False, stop=False)
                        for i in range(nT):
                            nc.tensor.matmul(
                                pss[i], ones_sb, bias_sb[:, dc * FC:(dc + 1) * FC],
                                start=False, stop=True)
                        for i in range(nT):
                            nc.scalar.activation(
                                gate[:, i, dc * FC:(dc + 1) * FC], pss[i], AF.Sigmoid)

                    if stop_after == "gate":
                        continue
                    # fused = hidden + gate*ctx; out = rmsnorm(fused) * norm_w
                    for i in range(nT):
                        hid_t = strm.tile([P, D], f32, tag="hid", bufs=2, name=f"hid{rep_blk}_{i}")
                        nc.sync.dma_start(hid_t, hid[n0 + i * P: n0 + (i + 1) * P, :])
                        fo = strm.tile([P, D], f32, tag="fo", bufs=2, name=f"fo{rep_blk}_{i}")
                        nc.vector.tensor_mul(fo, gate[:, i, :], ctxt[:, i, :])
                        nc.vector.tensor_add(fo, fo, hid_t)
                        if stop_after == "fused1":
                            nc.sync.dma_start(out[n0 + i * P: n0 + (i + 1) * P, :], fo)
                            continue
                        sq = strm.tile([P, D], f32, tag="hid", bufs=2, name=f"sq{rep_blk}_{i}")
                        ssq = sm.tile([P, 1], f32, tag="ssq", name=f"ssq{rep_blk}_{i}")
                        nc.scalar.activation(sq, fo, AF.Square, accum_out=ssq)
                        rstd = sm.tile([P, 1], f32, tag="rstd", name=f"rstd{rep_blk}_{i}")
                        nc.scalar.activation(rstd, ssq, AF.Sqrt, bias=eps_t, scale=1.0 / D)
                        nc.vector.reciprocal(rstd, rstd)
                        if stop_after == "fused2":
                            nc.sync.dma_start(out[n0 + i * P: n0 + (i + 1) * P, :], fo)
                            continue
                        nc.scalar.mul(fo, fo, rstd)
                        nc.vector.tensor_mul(fo, fo, nw_sb)
                        nc.sync.dma_start(out[n0 + i * P: n0 + (i + 1) * P, :], fo)

            hold_cm.__exit__(None, None, None)

    nc.compile()
    return nc


_PROG_CACHE = {}


def _get_program(key, **kw):
    if key not in _PROG_CACHE:
        _PROG_CACHE[key] = build_program(**kw)
    return _PROG_CACHE[key]


def prepare(inputs):
    """Build (prog, in_maps) for the 8-core SPMD run."""
    return _prepare(inputs["hidden_states"], inputs["memory"], inputs["W_mem"],
                    inputs["W_gate"], inputs["b_gate"], inputs["norm_w"])


def _prepare(hidden_states, memory, W_mem, W_gate, b_gate, norm_w):
    B, N, D = hidden_states.shape
    _, M, E = memory.shape
    NC = 8
    H = NC // B                      # N-splits per batch (2)
    BN = N // H                      # rows per core (2048)

    prog = _get_program(("full", BN, M, D, E), BN=BN, M=M, D=D, E=E)

    import ml_dtypes
    f32 = np.float32
    bf16 = ml_dtypes.bfloat16
    WmT = np.ascontiguousarray(W_mem.T).astype(bf16)
    WghT = np.ascontiguousarray(W_gate[:, :D].T).astype(bf16)
    WgcT = np.ascontiguousarray(W_gate[:, D:].T).astype(bf16)
    bg = np.ascontiguousarray(b_gate[None, :]).astype(bf16)
    nw = np.ascontiguousarray(norm_w[None, :], dtype=f32)
    ident = np.eye(P, dtype=f32).astype(bf16)
    ones = np.ones((1, P), dtype=bf16)

    in_maps = []
    for c in range(NC):
        b, h = c // H, c % H
        hs = hidden_states[b, h * BN:(h + 1) * BN, :]
        in_maps.append({
            "hiddenT": np.ascontiguousarray(hs.T).astype(bf16),
            "hidden": np.ascontiguousarray(hs, dtype=f32),
            "memT": np.ascontiguousarray(memory[b].T).astype(bf16),
            "WmT": WmT, "WghT": WghT, "WgcT": WgcT,
            "b_gate": bg, "norm_w": nw, "ident": ident, "ones": ones,
        })
    return prog, in_maps


def kernel(hidden_states, memory, W_mem, W_gate, b_gate, norm_w):
    from concourse.bass_utils import run_bass_kernel_spmd

    B, N, D = hidden_states.shape
    NC = 8
    H = NC // B
    BN = N // H
    prog, in_maps = _prepare(hidden_states, memory, W_mem, W_gate,
                             b_gate, norm_w)
    res = run_bass_kernel_spmd(prog, in_maps, core_ids=list(range(NC)))
    out = np.empty((B, N, D), dtype=np.float32)
    for c in range(NC):
        b, h = c // H, c % H
        out[b, h * BN:(h + 1) * BN, :] = res.results[c]["out"]
    return out



# revision 13
# speedup vs baseline: 1.5834x; 1.5834x over previous
"""MemoryGate kernel for Trainium2 (8 NeuronCores, SPMD), fp8 DoubleRow.

Math (per batch b):
    mp   = memory[b] @ W_mem.T                      [M, D]
    S    = hidden[b] @ mp.T / sqrt(D)               [N, M]
    A    = softmax(S, axis=-1)
    ctx  = A @ mp                                   [N, D]
    gate = sigmoid(hidden @ Wg_h.T + ctx @ Wg_c.T + b_gate)
    out  = rmsnorm(hidden + gate * ctx) * norm_w

Sharding: 8 cores = 4 batches x 2 N-halves. Each core computes mp for its
batch (duplicated across the pair) and processes N/2 = 2048 rows.

All big matmuls run fp8(e4m3) with perf_mode=DoubleRow: operands laid out
[128, ktiles, X] and sliced [:, 2t:2t+2, :] so each MM contracts K=256.
PSUM accumulates fp32. Softmax normalization is deferred: exp() writes
unnormalized fp8 attn weights (bias=-1 keeps the range within e4m3), the
fp32 row sums come from the activation accumulator, and 1/sum is applied
in the ctx PSUM drain (so ctx, ctxT and the gate see normalized values).

Layout strategy (per core, all [partition, free]):
    hT8     [D, BN] fp8 (host pre-transposed)  -> lhsT for scores / gate-G1
    memT8   [E, M], WmT8 [E, D] fp8            -> stage A operands
    mpT8    [D, M] fp8 (unscaled; 1/sqrt(D) folded into the exp scale)
    mp8     [M, D] fp8                          -> rhs for ctx
    scores PSUM [n-part, m-free]; exp -> attn8 fp8 (unnormalized)
    attn8 transposed on PE (fp8 128x128 blocks) -> attnT8 lhsT for ctx
    ctx drained with *1/rowsum -> ctx16 bf16; PE-transposed -> ctxT8 fp8
    gate rhs (Wgh/Wgc fp8) streamed from DRAM in [128,2,512] k-pair chunks
    b_gate added via a K=1 bf16 matmul into the same PSUM accumulation
    rmsnorm along free dim in [n-part, d-free] fp32 layout
"""

import math
import os
import sys

for _p in ("/opt/trn_rl_repo", "/root/.axon_site/_ro/trn_rl_repo"):
    if os.path.isdir(_p) and _p not in sys.path:
        sys.path.append(_p)

import numpy as np

P = 128


def build_program(BN=2048, M=2048, D=2048, E=1024, NB=512, FC=512,
                  stop_after=None):
    """Build the per-core Bass program. All shapes must divide evenly.

    stop_after: debug aid — truncate the program after a named phase
    ("A", "scores", "attnT", "ctx", "ctxT", "gate"); None = full kernel.
    """
    import concourse.tile as tile
    from concourse import bacc, mybir

    f32 = mybir.dt.float32
    bf16 = mybir.dt.bfloat16
    fp8 = mybir.dt.float8e4
    AF = mybir.ActivationFunctionType
    AX = mybir.AxisListType
    DR = mybir.MatmulPerfMode.DoubleRow

    kE, kD, mT, nT = E // P, D // P, M // P, NB // P
    NBLK = BN // NB
    mFC, dFC = M // FC, D // FC
    kEh, kDh, mTh = kE // 2, kD // 2, mT // 2
    SCALE = 1.0 / math.sqrt(D)
    EXPB = -2.5          # exp(s/sqrt(D) - 2.5): keeps fp8 attn well under 240
                         # (empirical max s ~ 6.6; e^{6.6-2.5} ~ 60, Inf at 7.98)
    EPS = 1e-6

    nc = bacc.Bacc("TRN2", target_bir_lowering=False, debug=False)

    hT = nc.dram_tensor("hiddenT", [D, BN], fp8, kind="ExternalInput")
    hid = nc.dram_tensor("hidden", [BN, D], f32, kind="ExternalInput")
    memT = nc.dram_tensor("memT", [E, M], fp8, kind="ExternalInput")
    WmT = nc.dram_tensor("WmT", [E, D], fp8, kind="ExternalInput")
    WghT = nc.dram_tensor("WghT", [D, D], fp8, kind="ExternalInput")
    WgcT = nc.dram_tensor("WgcT", [D, D], fp8, kind="ExternalInput")
    bg = nc.dram_tensor("b_gate", [1, D], bf16, kind="ExternalInput")
    nw = nc.dram_tensor("norm_w", [1, D], f32, kind="ExternalInput")
    id8d = nc.dram_tensor("ident8", [P, P], fp8, kind="ExternalInput")
    id16d = nc.dram_tensor("ident16", [P, P], bf16, kind="ExternalInput")
    oned = nc.dram_tensor("ones", [1, P], bf16, kind="ExternalInput")
    out = nc.dram_tensor("out", [BN, D], f32, kind="ExternalOutput")

    with tile.TileContext(nc) as tc:
        with tc.tile_pool(name="const", bufs=1) as const:
            ident8 = const.tile([P, P], fp8, tag="id8", name="id8_sb")
            nc.sync.dma_start(ident8, id8d[:])
            ident16 = const.tile([P, P], bf16, tag="id16", name="id16_sb")
            nc.sync.dma_start(ident16, id16d[:])
            ones_sb = const.tile([1, P], bf16, tag="ones", name="ones_sb")
            nc.sync.dma_start(ones_sb, oned[:])
            bias_sb = const.tile([1, D], bf16, tag="bias", name="bias_sb")
            nc.sync.dma_start(bias_sb, bg[:])
            nw_sb = const.tile([P, D], f32, tag="nw", name="nw_sb")
            nc.gpsimd.dma_start(nw_sb, nw[:].partition_broadcast(P))
            eps_t = const.tile([P, 1], f32, tag="eps", name="eps_sb")
            nc.vector.memset(eps_t, EPS)
            expb_t = const.tile([P, 1], f32, tag="expb", name="expb_sb")
            nc.vector.memset(expb_t, EXPB)

            # mpT8 + mp8 stay resident in SBUF for the whole kernel (8 MiB)
            hold_cm = tc.tile_pool(name="hold", bufs=1)
            hold = hold_cm.__enter__()
            mpT8 = hold.tile([P, kD, M], fp8, tag="mpT", name="mpT_sb")
            mp8 = hold.tile([P, mT, D], fp8, tag="mp", name="mp_sb")

            # ---------------- Stage A: mpT8 and mp8 (both unscaled) --------
            with (
                tc.tile_pool(name="a_in", bufs=1) as a_in,
                tc.tile_pool(name="a_ps", bufs=4, space="PSUM") as a_ps,
            ):
                memT_sb = a_in.tile([P, kE, M], fp8, tag="memT", name="memT_sb")
                WmT_sb = a_in.tile([P, kE, D], fp8, tag="WmT", name="WmT_sb")
                for k in range(kE):
                    nc.sync.dma_start(memT_sb[:, k, :], memT[k * P:(k + 1) * P, :])
                    nc.sync.dma_start(WmT_sb[:, k, :], WmT[k * P:(k + 1) * P, :])
                # A1: mpT[d, m] = sum_e WmT[e, d] * memT[e, m]
                for dp in range(kD):
                    for mc in range(mFC):
                        ps = a_ps.tile([P, FC], f32, tag="ps", name=f"a1ps{dp}_{mc}")
                        for t in range(kEh):
                            nc.tensor.matmul(
                                ps,
                                WmT_sb[:, 2 * t:2 * t + 2, dp * P:(dp + 1) * P],
                                memT_sb[:, 2 * t:2 * t + 2, mc * FC:(mc + 1) * FC],
                                start=(t == 0), stop=(t == kEh - 1),
                                perf_mode=DR,
                            )
                        nc.scalar.copy(mpT8[:, dp, mc * FC:(mc + 1) * FC], ps)
                # A2: mp[m, d] = sum_e memT[e, m] * WmT[e, d]
                for mp_ in range(mT):
                    for dc in range(dFC):
                        ps = a_ps.tile([P, FC], f32, tag="ps", name=f"a2ps{mp_}_{dc}")
                        for t in range(kEh):
                            nc.tensor.matmul(
                                ps,
                                memT_sb[:, 2 * t:2 * t + 2, mp_ * P:(mp_ + 1) * P],
                                WmT_sb[:, 2 * t:2 * t + 2, dc * FC:(dc + 1) * FC],
                                start=(t == 0), stop=(t == kEh - 1),
                                perf_mode=DR,
                            )
                        nc.scalar.copy(mp8[:, mp_, dc * FC:(dc + 1) * FC], ps)

            # ---------------- Stage B: per N-block pipeline -----------------
            with (
                tc.tile_pool(name="b_big", bufs=1) as bb,
                tc.tile_pool(name="b_strm", bufs=6) as strm,
                tc.tile_pool(name="b_sm", bufs=2) as sm,
                tc.tile_pool(name="b_ps", bufs=6, space="PSUM") as bps,
            ):
                for blk in range(NBLK):
                    n0 = blk * NB
                    hT_sb = bb.tile([P, kD, NB], fp8, tag="hT", bufs=2,
                                    name=f"hT{blk}")
                    for k in range(kD):
                        nc.sync.dma_start(hT_sb[:, k, :], hT[k * P:(k + 1) * P, n0:n0 + NB])

                    if stop_after == "A":
                        if blk == 0:
                            for mt in range(mT):
                                cp = strm.tile([P, D], f32, tag="dbg", bufs=2,
                                               name=f"dbgA{mt}")
                                nc.vector.tensor_copy(cp, mp8[:, mt, :])
                                nc.sync.dma_start(out[mt * P:(mt + 1) * P, :], cp)
                        continue
                    # scores + exp (unnormalized attn, fp8) + row-chunk sums
                    attn = bb.tile([P, nT, M], bf16, tag="attn", name=f"attn{blk}")
                    sums = sm.tile([P, nT * mFC], f32, tag="sums", name=f"sums{blk}")
                    rs = sm.tile([P, nT], f32, tag="rs", name=f"rs{blk}")
                    for mc in range(mFC):
                        pss = [bps.tile([P, FC], f32, tag="ps", name=f"sc{blk}_{mc}_{i}")
                               for i in range(nT)]
                        for t in range(kDh):
                            for i in range(nT):
                                nc.tensor.matmul(
                                    pss[i],
                                    hT_sb[:, 2 * t:2 * t + 2, i * P:(i + 1) * P],
                                    mpT8[:, 2 * t:2 * t + 2, mc * FC:(mc + 1) * FC],
                                    start=(t == 0), stop=(t == kDh - 1),
                                    perf_mode=DR,
                                )
                        for i in range(nT):
                            nc.scalar.activation(
                                attn[:, i, mc * FC:(mc + 1) * FC], pss[i], AF.Exp,
                                scale=SCALE, bias=expb_t,
                                accum_out=sums[:, i * mFC + mc: i * mFC + mc + 1])
                    # softmax denominators (normalization deferred to ctx drain)
                    for i in range(nT):
                        nc.vector.reduce_sum(
                            out=rs[:, i:i + 1], in_=sums[:, i * mFC:(i + 1) * mFC], axis=AX.X)
                    nc.vector.reciprocal(rs, rs)

                    if stop_after == "scores":
                        for i in range(nT):
                            cp = strm.tile([P, M], f32, tag="dbg", bufs=2,
                                           name=f"dbgS{blk}_{i}")
                            nc.vector.tensor_copy(cp, attn[:, i, :])
                            nc.sync.dma_start(out[n0 + i * P:n0 + (i + 1) * P, :], cp)
                        continue
                    # transpose attn -> attnT (fp8)
                    attnT = bb.tile([P, mT, NB], fp8, tag="attnT", name=f"attnT{blk}")
                    for mt in range(mT):
                        tp = bps.tile([P, NB], bf16, tag="ps", name=f"tpa{blk}_{mt}")
                        for i in range(nT):
                            nc.tensor.transpose(
                                tp[:, i * P:(i + 1) * P], attn[:, i, mt * P:(mt + 1) * P], ident16)
                        nc.vector.tensor_copy(attnT[:, mt, :], tp)

                    if stop_after == "attnT":
                        continue
                    # ctx = (attn @ mp) * 1/rowsum  (normalization in the drain)
                    ctxt = bb.tile([P, nT, D], bf16, tag="ctx", name=f"ctx{blk}")
                    for dc in range(dFC):
                        pss = [bps.tile([P, FC], f32, tag="ps", name=f"cx{blk}_{dc}_{i}")
                               for i in range(nT)]
                        for t in range(mTh):
                            for i in range(nT):
                                nc.tensor.matmul(
                                    pss[i],
                                    attnT[:, 2 * t:2 * t + 2, i * P:(i + 1) * P],
                                    mp8[:, 2 * t:2 * t + 2, dc * FC:(dc + 1) * FC],
                                    start=(t == 0), stop=(t == mTh - 1),
                                    perf_mode=DR,
                                )
                        for i in range(nT):
                            nc.scalar.mul(
                                ctxt[:, i, dc * FC:(dc + 1) * FC], pss[i], rs[:, i:i + 1])

                    if stop_after == "ctx":
                        for i in range(nT):
                            cp = strm.tile([P, D], f32, tag="dbg", bufs=2,
                                           name=f"dbgC{blk}_{i}")
                            nc.vector.tensor_copy(cp, ctxt[:, i, :])
                            nc.sync.dma_start(out[n0 + i * P:n0 + (i + 1) * P, :], cp)
                        continue
                    # transpose ctx -> ctxT (bf16 -> fp8 in the DVE copy)
                    ctxT = bb.tile([P, kD, NB], fp8, tag="ctxT", name=f"ctxT{blk}")
                    for dt_ in range(kD):
                        tp = bps.tile([P, NB], bf16, tag="ps", name=f"tpc{blk}_{dt_}")
                        for i in range(nT):
                            nc.tensor.transpose(
                                tp[:, i * P:(i + 1) * P], ctxt[:, i, dt_ * P:(dt_ + 1) * P], ident16)
                        nc.vector.tensor_copy(ctxT[:, dt_, :], tp)

                    if stop_after == "ctxT":
                        continue
                    # gate = sigmoid(hidden @ WghT + ctx @ WgcT + b_gate)
                    gate = bb.tile([P, nT, D], bf16, tag="gate", name=f"gate{blk}")
                    for dc in range(dFC):
                        pss = [bps.tile([P, FC], f32, tag="ps", name=f"gt{blk}_{dc}_{i}")
                               for i in range(nT)]
                        for t in range(kDh):
                            ch = strm.tile([P, 2, FC], fp8, tag="rhs", name=f"g1ch{blk}_{dc}_{t}")
                            nc.sync.dma_start(
                                ch[:, 0, :], WghT[2 * t * P:(2 * t + 1) * P, dc * FC:(dc + 1) * FC])
                            nc.sync.dma_start(
                                ch[:, 1, :], WghT[(2 * t + 1) * P:(2 * t + 2) * P, dc * FC:(dc + 1) * FC])
                            for i in range(nT):
                                nc.tensor.matmul(
                                    pss[i],
                                    hT_sb[:, 2 * t:2 * t + 2, i * P:(i + 1) * P], ch,
                                    start=(t == 0), stop=False, perf_mode=DR)
                        for t in range(kDh):
                            ch = strm.tile([P, 2, FC], fp8, tag="rhs", name=f"g2ch{blk}_{dc}_{t}")
                            nc.sync.dma_start(
                                ch[:, 0, :], WgcT[2 * t * P:(2 * t + 1) * P, dc * FC:(dc + 1) * FC])
                            nc.sync.dma_start(
                                ch[:, 1, :], WgcT[(2 * t + 1) * P:(2 * t + 2) * P, dc * FC:(dc + 1) * FC])
                            for i in range(nT):
                                nc.tensor.matmul(
                                    pss[i],
                                    ctxT[:, 2 * t:2 * t + 2, i * P:(i + 1) * P], ch,
                                    start=False, stop=False, perf_mode=DR)
                        for i in range(nT):
                            nc.tensor.matmul(
                                pss[i], ones_sb, bias_sb[:, dc * FC:(dc + 1) * FC],
                                start=False, stop=True)
                        for i in range(nT):
                            nc.scalar.activation(
                                gate[:, i, dc * FC:(dc + 1) * FC], pss[i], AF.Sigmoid)

                    if stop_after == "gate":
                        for i in range(nT):
                            cp = strm.tile([P, D], f32, tag="dbg", bufs=2,
                                           name=f"dbgG{blk}_{i}")
                            nc.vector.tensor_copy(cp, gate[:, i, :])
                            nc.sync.dma_start(out[n0 + i * P:n0 + (i + 1) * P, :], cp)
                        continue
                    # fused = hidden + gate*ctx; out = rmsnorm(fused) * norm_w
                    for i in range(nT):
                        hid_t = strm.tile([P, D], f32, tag="hid", bufs=2, name=f"hid{blk}_{i}")
                        nc.sync.dma_start(hid_t, hid[n0 + i * P: n0 + (i + 1) * P, :])
                        fo = strm.tile([P, D], f32, tag="fo", bufs=2, name=f"fo{blk}_{i}")
                        nc.vector.tensor_mul(fo, gate[:, i, :], ctxt[:, i, :])
                        nc.vector.tensor_add(fo, fo, hid_t)
                        sq = strm.tile([P, D], f32, tag="hid", bufs=2, name=f"sq{blk}_{i}")
                        ssq = sm.tile([P, 1], f32, tag="ssq", name=f"ssq{blk}_{i}")
                        nc.scalar.activation(sq, fo, AF.Square, accum_out=ssq)
                        rstd = sm.tile([P, 1], f32, tag="rstd", name=f"rstd{blk}_{i}")
                        nc.scalar.activation(rstd, ssq, AF.Sqrt, bias=eps_t, scale=1.0 / D)
                        nc.vector.reciprocal(rstd, rstd)
                        nc.scalar.mul(fo, fo, rstd)
                        nc.vector.tensor_mul(fo, fo, nw_sb)
                        nc.sync.dma_start(out[n0 + i * P: n0 + (i + 1) * P, :], fo)

            hold_cm.__exit__(None, None, None)

    nc.compile()
    return nc


_PROG_CACHE = {}


def _get_program(key, **kw):
    if key not in _PROG_CACHE:
        _PROG_CACHE[key] = build_program(**kw)
    return _PROG_CACHE[key]


def prepare(inputs):
    """Build (prog, in_maps) for the 8-core SPMD run."""
    return _prepare(inputs["hidden_states"], inputs["memory"], inputs["W_mem"],
                    inputs["W_gate"], inputs["b_gate"], inputs["norm_w"])


def _prepare(hidden_states, memory, W_mem, W_gate, b_gate, norm_w):
    B, N, D = hidden_states.shape
    _, M, E = memory.shape
    NC = 8
    H = NC // B                      # N-splits per batch (2)
    BN = N // H                      # rows per core (2048)

    prog = _get_program(("fp8", BN, M, D, E), BN=BN, M=M, D=D, E=E)

    import ml_dtypes
    f32 = np.float32
    bf16 = ml_dtypes.bfloat16
    fp8 = ml_dtypes.float8_e4m3
    WmT = np.ascontiguousarray(W_mem.T).astype(fp8)
    WghT = np.ascontiguousarray(W_gate[:, :D].T).astype(fp8)
    WgcT = np.ascontiguousarray(W_gate[:, D:].T).astype(fp8)
    bg = np.ascontiguousarray(b_gate[None, :]).astype(bf16)
    nw = np.ascontiguousarray(norm_w[None, :], dtype=f32)
    ident8 = np.eye(P, dtype=f32).astype(fp8)
    ident16 = np.eye(P, dtype=f32).astype(bf16)
    ones = np.ones((1, P), dtype=bf16)

    in_maps = []
    for c in range(NC):
        b, h = c // H, c % H
        hs = hidden_states[b, h * BN:(h + 1) * BN, :]
        in_maps.append({
            "hiddenT": np.ascontiguousarray(hs.T).astype(fp8),
            "hidden": np.ascontiguousarray(hs, dtype=f32),
            "memT": np.ascontiguousarray(memory[b].T).astype(fp8),
            "WmT": WmT, "WghT": WghT, "WgcT": WgcT,
            "b_gate": bg, "norm_w": nw,
            "ident8": ident8, "ident16": ident16, "ones": ones,
        })
    return prog, in_maps


def kernel(hidden_states, memory, W_mem, W_gate, b_gate, norm_w):
    from concourse.bass_utils import run_bass_kernel_spmd

    B, N, D = hidden_states.shape
    NC = 8
    H = NC // B
    BN = N // H
    prog, in_maps = _prepare(hidden_states, memory, W_mem, W_gate,
                             b_gate, norm_w)
    res = run_bass_kernel_spmd(prog, in_maps, core_ids=list(range(NC)))
    out = np.empty((B, N, D), dtype=np.float32)
    for c in range(NC):
        b, h = c // H, c % H
        out[b, h * BN:(h + 1) * BN, :] = res.results[c]["out"]
    return out


# revision 16
# speedup vs baseline: 1.9378x; 1.2238x over previous
"""MemoryGate kernel for Trainium2 (8 NeuronCores, SPMD), fp8 DoubleRow.

Math (per batch b):
    mp   = memory[b] @ W_mem.T                      [M, D]
    S    = hidden[b] @ mp.T / sqrt(D)               [N, M]
    A    = softmax(S, axis=-1)
    ctx  = A @ mp                                   [N, D]
    gate = sigmoid(hidden @ Wg_h.T + ctx @ Wg_c.T + b_gate)
    out  = rmsnorm(hidden + gate * ctx) * norm_w

Sharding: 8 cores = 4 batches x 2 N-halves. Each core computes mp for its
batch (duplicated across the pair) and processes N/2 = 2048 rows.

All big matmuls run fp8(e4m3) with perf_mode=DoubleRow: operands laid out
[128, ktiles, X] and sliced [:, 2t:2t+2, :] so each MM contracts K=256.
PSUM accumulates fp32. Softmax normalization is deferred: exp() writes
unnormalized fp8 attn weights (bias=-1 keeps the range within e4m3), the
fp32 row sums come from the activation accumulator, and 1/sum is applied
in the ctx PSUM drain (so ctx, ctxT and the gate see normalized values).

Layout strategy (per core, all [partition, free]):
    hT8     [D, BN] fp8 (host pre-transposed)  -> lhsT for scores / gate-G1
    memT8   [E, M], WmT8 [E, D] fp8            -> stage A operands
    mpT8    [D, M] fp8 (unscaled; 1/sqrt(D) folded into the exp scale)
    mp8     [M, D] fp8                          -> rhs for ctx
    scores PSUM [n-part, m-free]; exp -> attn8 fp8 (unnormalized)
    attn8 transposed on PE (fp8 128x128 blocks) -> attnT8 lhsT for ctx
    ctx drained with *1/rowsum -> ctx16 bf16; PE-transposed -> ctxT8 fp8
    gate rhs (Wgh/Wgc fp8) streamed from DRAM in [128,2,512] k-pair chunks
    b_gate added via a K=1 bf16 matmul into the same PSUM accumulation
    rmsnorm along free dim in [n-part, d-free] fp32 layout
"""

import math
import os
import sys

for _p in ("/opt/trn_rl_repo", "/root/.axon_site/_ro/trn_rl_repo"):
    if os.path.isdir(_p) and _p not in sys.path:
        sys.path.append(_p)

import numpy as np

P = 128


def build_program(BN=2048, M=2048, D=2048, E=1024, NB=512, FC=512,
                  stop_after=None):
    """Build the per-core Bass program. All shapes must divide evenly.

    stop_after: debug aid — truncate the program after a named phase
    ("A", "scores", "attnT", "ctx", "ctxT", "gate"); None = full kernel.
    """
    import concourse.tile as tile
    from concourse import bacc, mybir

    f32 = mybir.dt.float32
    bf16 = mybir.dt.bfloat16
    fp8 = mybir.dt.float8e4
    AF = mybir.ActivationFunctionType
    AX = mybir.AxisListType
    DR = mybir.MatmulPerfMode.DoubleRow

    kE, kD, mT, nT = E // P, D // P, M // P, NB // P
    NBLK = BN // NB
    mFC, dFC = M // FC, D // FC
    kEh, kDh, mTh = kE // 2, kD // 2, mT // 2
    SCALE = 1.0 / math.sqrt(D)
    EXPB = -2.5          # exp(s/sqrt(D) - 2.5): keeps fp8 attn well under 240
                         # (empirical max s ~ 6.6; e^{6.6-2.5} ~ 60, Inf at 7.98)
    EPS = 1e-6

    nc = bacc.Bacc("TRN2", target_bir_lowering=False, debug=False)

    hT = nc.dram_tensor("hiddenT", [D, BN], fp8, kind="ExternalInput")
    hid = nc.dram_tensor("hidden", [BN, D], f32, kind="ExternalInput")
    memT = nc.dram_tensor("memT", [E, M], fp8, kind="ExternalInput")
    WmT = nc.dram_tensor("WmT", [E, D], fp8, kind="ExternalInput")
    WghT = nc.dram_tensor("WghT", [D, D], fp8, kind="ExternalInput")
    WgcT = nc.dram_tensor("WgcT", [D, D], fp8, kind="ExternalInput")
    bg = nc.dram_tensor("b_gate", [1, D], bf16, kind="ExternalInput")
    nw = nc.dram_tensor("norm_w", [1, D], f32, kind="ExternalInput")
    id8d = nc.dram_tensor("ident8", [P, P], fp8, kind="ExternalInput")
    id16d = nc.dram_tensor("ident16", [P, P], bf16, kind="ExternalInput")
    oned = nc.dram_tensor("ones", [1, P], bf16, kind="ExternalInput")
    out = nc.dram_tensor("out", [BN, D], f32, kind="ExternalOutput")

    with tile.TileContext(nc) as tc:
        with tc.tile_pool(name="const", bufs=1) as const:
            ident8 = const.tile([P, P], fp8, tag="id8", name="id8_sb")
            nc.sync.dma_start(ident8, id8d[:])
            ident16 = const.tile([P, P], bf16, tag="id16", name="id16_sb")
            nc.sync.dma_start(ident16, id16d[:])
            ones_sb = const.tile([1, P], bf16, tag="ones", name="ones_sb")
            nc.sync.dma_start(ones_sb, oned[:])
            bias_sb = const.tile([1, D], bf16, tag="bias", name="bias_sb")
            nc.sync.dma_start(bias_sb, bg[:])
            nw_sb = const.tile([P, D], f32, tag="nw", name="nw_sb")
            nc.gpsimd.dma_start(nw_sb, nw[:].partition_broadcast(P))
            eps_t = const.tile([P, 1], f32, tag="eps", name="eps_sb")
            nc.vector.memset(eps_t, EPS)
            expb_t = const.tile([P, 1], f32, tag="expb", name="expb_sb")
            nc.vector.memset(expb_t, EXPB)

            # mpT8 + mp8 stay resident in SBUF for the whole kernel (8 MiB)
            hold_cm = tc.tile_pool(name="hold", bufs=1)
            hold = hold_cm.__enter__()
            mpT8 = hold.tile([P, kD, M], fp8, tag="mpT", name="mpT_sb")
            mp8 = hold.tile([P, mT, D], fp8, tag="mp", name="mp_sb")

            # ---------------- Stage A: mpT8 and mp8 (both unscaled) --------
            with (
                tc.tile_pool(name="a_in", bufs=1) as a_in,
                tc.tile_pool(name="a_ps", bufs=4, space="PSUM") as a_ps,
            ):
                # per-k-pair tiles so the first matmul starts after the
                # first pair lands instead of after all of E
                memT_p = [a_in.tile([P, 2, M], fp8, tag=f"memT{t}",
                                    name=f"memT_sb{t}") for t in range(kEh)]
                WmT_p = [a_in.tile([P, 2, D], fp8, tag=f"WmT{t}",
                                   name=f"WmT_sb{t}") for t in range(kEh)]
                for t in range(kEh):
                    for s in range(2):
                        k = 2 * t + s
                        nc.sync.dma_start(memT_p[t][:, s, :], memT[k * P:(k + 1) * P, :])
                        nc.sync.dma_start(WmT_p[t][:, s, :], WmT[k * P:(k + 1) * P, :])
                # A1: mpT[d, m] = sum_e WmT[e, d] * memT[e, m]
                for dp in range(kD):
                    for mc in range(mFC):
                        ps = a_ps.tile([P, FC], f32, tag="ps", name=f"a1ps{dp}_{mc}")
                        for t in range(kEh):
                            nc.tensor.matmul(
                                ps,
                                WmT_p[t][:, :, dp * P:(dp + 1) * P],
                                memT_p[t][:, :, mc * FC:(mc + 1) * FC],
                                start=(t == 0), stop=(t == kEh - 1),
                                perf_mode=DR,
                            )
                        nc.scalar.copy(mpT8[:, dp, mc * FC:(mc + 1) * FC], ps)
                # A2: mp[m, d] = sum_e memT[e, m] * WmT[e, d]
                for mp_ in range(mT):
                    for dc in range(dFC):
                        ps = a_ps.tile([P, FC], f32, tag="ps", name=f"a2ps{mp_}_{dc}")
                        for t in range(kEh):
                            nc.tensor.matmul(
                                ps,
                                memT_p[t][:, :, mp_ * P:(mp_ + 1) * P],
                                WmT_p[t][:, :, dc * FC:(dc + 1) * FC],
                                start=(t == 0), stop=(t == kEh - 1),
                                perf_mode=DR,
                            )
                        nc.scalar.copy(mp8[:, mp_, dc * FC:(dc + 1) * FC], ps)

            # ---------------- Stage B: per N-block pipeline -----------------
            with (
                tc.tile_pool(name="b_big", bufs=1) as bb,
                tc.tile_pool(name="b_strm", bufs=8) as strm,
                tc.tile_pool(name="b_sm", bufs=2) as sm,
                tc.tile_pool(name="b_ps", bufs=8, space="PSUM") as bps,
            ):
                for blk in range(NBLK):
                    n0 = blk * NB
                    hT_sb = bb.tile([P, kD, NB], fp8, tag="hT", bufs=2,
                                    name=f"hT{blk}")
                    for k in range(kD):
                        nc.sync.dma_start(hT_sb[:, k, :], hT[k * P:(k + 1) * P, n0:n0 + NB])

                    if stop_after == "A":
                        if blk == 0:
                            for mt in range(mT):
                                cp = strm.tile([P, D], f32, tag="dbg", bufs=2,
                                               name=f"dbgA{mt}")
                                nc.vector.tensor_copy(cp, mp8[:, mt, :])
                                nc.sync.dma_start(out[mt * P:(mt + 1) * P, :], cp)
                        continue
                    # scores + exp (unnormalized attn, fp8) + row-chunk sums
                    attn = bb.tile([P, nT, M], bf16, tag="attn", name=f"attn{blk}")
                    sums = sm.tile([P, nT * mFC], f32, tag="sums", name=f"sums{blk}")
                    rs = sm.tile([P, nT], f32, tag="rs", name=f"rs{blk}")
                    for mc in range(mFC):
                        pss = [bps.tile([P, FC], f32, tag="ps", name=f"sc{blk}_{mc}_{i}")
                               for i in range(nT)]
                        for t in range(kDh):
                            for i in range(nT):
                                nc.tensor.matmul(
                                    pss[i],
                                    hT_sb[:, 2 * t:2 * t + 2, i * P:(i + 1) * P],
                                    mpT8[:, 2 * t:2 * t + 2, mc * FC:(mc + 1) * FC],
                                    start=(t == 0), stop=(t == kDh - 1),
                                    perf_mode=DR,
                                )
                        for i in range(nT):
                            nc.scalar.activation(
                                attn[:, i, mc * FC:(mc + 1) * FC], pss[i], AF.Exp,
                                scale=SCALE, bias=expb_t,
                                accum_out=sums[:, i * mFC + mc: i * mFC + mc + 1])
                    # softmax denominators (normalization deferred to ctx drain)
                    for i in range(nT):
                        nc.vector.reduce_sum(
                            out=rs[:, i:i + 1], in_=sums[:, i * mFC:(i + 1) * mFC], axis=AX.X)
                    nc.vector.reciprocal(rs, rs)

                    if stop_after == "scores":
                        for i in range(nT):
                            cp = strm.tile([P, M], f32, tag="dbg", bufs=2,
                                           name=f"dbgS{blk}_{i}")
                            nc.vector.tensor_copy(cp, attn[:, i, :])
                            nc.sync.dma_start(out[n0 + i * P:n0 + (i + 1) * P, :], cp)
                        continue
                    # transpose attn -> attnT (fp8)
                    attnT = bb.tile([P, mT, NB], fp8, tag="attnT", name=f"attnT{blk}")
                    for mt in range(mT):
                        tp = bps.tile([P, NB], bf16, tag="ps", name=f"tpa{blk}_{mt}")
                        for i in range(nT):
                            nc.tensor.transpose(
                                tp[:, i * P:(i + 1) * P], attn[:, i, mt * P:(mt + 1) * P], ident16)
                        nc.vector.tensor_copy(attnT[:, mt, :], tp)

                    if stop_after == "attnT":
                        continue
                    # ctx = (attn @ mp) * 1/rowsum  (normalization in the drain)
                    ctxt = bb.tile([P, nT, D], bf16, tag="ctx", name=f"ctx{blk}")
                    for dc in range(dFC):
                        pss = [bps.tile([P, FC], f32, tag="ps", name=f"cx{blk}_{dc}_{i}")
                               for i in range(nT)]
                        for t in range(mTh):
                            for i in range(nT):
                                nc.tensor.matmul(
                                    pss[i],
                                    attnT[:, 2 * t:2 * t + 2, i * P:(i + 1) * P],
                                    mp8[:, 2 * t:2 * t + 2, dc * FC:(dc + 1) * FC],
                                    start=(t == 0), stop=(t == mTh - 1),
                                    perf_mode=DR,
                                )
                        for i in range(nT):
                            nc.scalar.mul(
                                ctxt[:, i, dc * FC:(dc + 1) * FC], pss[i], rs[:, i:i + 1])

                    if stop_after == "ctx":
                        for i in range(nT):
                            cp = strm.tile([P, D], f32, tag="dbg", bufs=2,
                                           name=f"dbgC{blk}_{i}")
                            nc.vector.tensor_copy(cp, ctxt[:, i, :])
                            nc.sync.dma_start(out[n0 + i * P:n0 + (i + 1) * P, :], cp)
                        continue
                    # transpose ctx -> ctxT (bf16 -> fp8 in the DVE copy)
                    ctxT = bb.tile([P, kD, NB], fp8, tag="ctxT", name=f"ctxT{blk}")
                    for dt_ in range(kD):
                        tp = bps.tile([P, NB], bf16, tag="ps", name=f"tpc{blk}_{dt_}")
                        for i in range(nT):
                            nc.tensor.transpose(
                                tp[:, i * P:(i + 1) * P], ctxt[:, i, dt_ * P:(dt_ + 1) * P], ident16)
                        nc.vector.tensor_copy(ctxT[:, dt_, :], tp)

                    if stop_after == "ctxT":
                        continue
                    # gate = sigmoid(hidden @ WghT + ctx @ WgcT + b_gate)
                    gate = bb.tile([P, nT, D], bf16, tag="gate", name=f"gate{blk}")
                    for dc in range(dFC):
                        pss = [bps.tile([P, FC], f32, tag="ps", name=f"gt{blk}_{dc}_{i}")
                               for i in range(nT)]
                        for t in range(kDh):
                            ch = strm.tile([P, 2, FC], fp8, tag="rhs", name=f"g1ch{blk}_{dc}_{t}")
                            nc.sync.dma_start(
                                ch[:, 0, :], WghT[2 * t * P:(2 * t + 1) * P, dc * FC:(dc + 1) * FC])
                            nc.sync.dma_start(
                                ch[:, 1, :], WghT[(2 * t + 1) * P:(2 * t + 2) * P, dc * FC:(dc + 1) * FC])
                            for i in range(nT):
                                nc.tensor.matmul(
                                    pss[i],
                                    hT_sb[:, 2 * t:2 * t + 2, i * P:(i + 1) * P], ch,
                                    start=(t == 0), stop=False, perf_mode=DR)
                        for t in range(kDh):
                            ch = strm.tile([P, 2, FC], fp8, tag="rhs", name=f"g2ch{blk}_{dc}_{t}")
                            nc.sync.dma_start(
                                ch[:, 0, :], WgcT[2 * t * P:(2 * t + 1) * P, dc * FC:(dc + 1) * FC])
                            nc.sync.dma_start(
                                ch[:, 1, :], WgcT[(2 * t + 1) * P:(2 * t + 2) * P, dc * FC:(dc + 1) * FC])
                            for i in range(nT):
                                nc.tensor.matmul(
                                    pss[i],
                                    ctxT[:, 2 * t:2 * t + 2, i * P:(i + 1) * P], ch,
                                    start=False, stop=False, perf_mode=DR)
                        for i in range(nT):
                            nc.tensor.matmul(
                                pss[i], ones_sb, bias_sb[:, dc * FC:(dc + 1) * FC],
                                start=False, stop=True)
                        for i in range(nT):
                            nc.scalar.activation(
                                gate[:, i, dc * FC:(dc + 1) * FC], pss[i], AF.Sigmoid)

                    if stop_after == "gate":
                        for i in range(nT):
                            cp = strm.tile([P, D], f32, tag="dbg", bufs=2,
                                           name=f"dbgG{blk}_{i}")
                            nc.vector.tensor_copy(cp, gate[:, i, :])
                            nc.sync.dma_start(out[n0 + i * P:n0 + (i + 1) * P, :], cp)
                        continue
                    # fused = hidden + gate*ctx; out = rmsnorm(fused) * norm_w
                    # chunked along D so it pipelines under the gate matmuls
                    for i in range(nT):
                        hid_t = strm.tile([P, D], f32, tag="hid", bufs=2, name=f"hid{blk}_{i}")
                        nc.sync.dma_start(hid_t, hid[n0 + i * P: n0 + (i + 1) * P, :])
                        fo = strm.tile([P, D], f32, tag="fo", bufs=2, name=f"fo{blk}_{i}")
                        sq = strm.tile([P, FC], f32, tag="sq", bufs=2, name=f"sq{blk}_{i}")
                        ssqp = sm.tile([P, dFC], f32, tag="ssq", name=f"ssqp{blk}_{i}")
                        for dc in range(dFC):
                            cs = slice(dc * FC, (dc + 1) * FC)
                            nc.vector.tensor_mul(fo[:, cs], gate[:, i, cs], ctxt[:, i, cs])
                            nc.vector.tensor_add(fo[:, cs], fo[:, cs], hid_t[:, cs])
                            nc.scalar.activation(sq, fo[:, cs], AF.Square,
                                                 accum_out=ssqp[:, dc:dc + 1])
                        ssq = sm.tile([P, 1], f32, tag="ssq", name=f"ssq{blk}_{i}")
                        nc.vector.reduce_sum(out=ssq, in_=ssqp, axis=AX.X)
                        rstd = sm.tile([P, 1], f32, tag="rstd", name=f"rstd{blk}_{i}")
                        nc.scalar.activation(rstd, ssq, AF.Sqrt, bias=eps_t, scale=1.0 / D)
                        nc.vector.reciprocal(rstd, rstd)
                        for dc in range(dFC):
                            cs = slice(dc * FC, (dc + 1) * FC)
                            nc.scalar.mul(fo[:, cs], fo[:, cs], rstd)
                            nc.vector.tensor_mul(fo[:, cs], fo[:, cs], nw_sb[:, cs])
                            nc.sync.dma_start(out[n0 + i * P: n0 + (i + 1) * P, cs], fo[:, cs])

            hold_cm.__exit__(None, None, None)

    nc.compile()
    return nc


_PROG_CACHE = {}


def _get_program(key, **kw):
    if key not in _PROG_CACHE:
        _PROG_CACHE[key] = build_program(**kw)
    return _PROG_CACHE[key]


def prepare(inputs):
    """Build (prog, in_maps) for the 8-core SPMD run."""
    return _prepare(inputs["hidden_states"], inputs["memory"], inputs["W_mem"],
                    inputs["W_gate"], inputs["b_gate"], inputs["norm_w"])


def _prepare(hidden_states, memory, W_mem, W_gate, b_gate, norm_w):
    B, N, D = hidden_states.shape
    _, M, E = memory.shape
    NC = 8
    H = NC // B                      # N-splits per batch (2)
    BN = N // H                      # rows per core (2048)

    prog = _get_program(("fp8", BN, M, D, E), BN=BN, M=M, D=D, E=E)

    import ml_dtypes
    f32 = np.float32
    bf16 = ml_dtypes.bfloat16
    fp8 = ml_dtypes.float8_e4m3
    WmT = np.ascontiguousarray(W_mem.T).astype(fp8)
    WghT = np.ascontiguousarray(W_gate[:, :D].T).astype(fp8)
    WgcT = np.ascontiguousarray(W_gate[:, D:].T).astype(fp8)
    bg = np.ascontiguousarray(b_gate[None, :]).astype(bf16)
    nw = np.ascontiguousarray(norm_w[None, :], dtype=f32)
    ident8 = np.eye(P, dtype=f32).astype(fp8)
    ident16 = np.eye(P, dtype=f32).astype(bf16)
    ones = np.ones((1, P), dtype=bf16)

    in_maps = []
    for c in range(NC):
        b, h = c // H, c % H
        hs = hidden_states[b, h * BN:(h + 1) * BN, :]
        in_maps.append({
            "hiddenT": np.ascontiguousarray(hs.T).astype(fp8),
            "hidden": np.ascontiguousarray(hs, dtype=f32),
            "memT": np.ascontiguousarray(memory[b].T).astype(fp8),
            "WmT": WmT, "WghT": WghT, "WgcT": WgcT,
            "b_gate": bg, "norm_w": nw,
            "ident8": ident8, "ident16": ident16, "ones": ones,
        })
    return prog, in_maps


def kernel(hidden_states, memory, W_mem, W_gate, b_gate, norm_w):
    from concourse.bass_utils import run_bass_kernel_spmd

    B, N, D = hidden_states.shape
    NC = 8
    H = NC // B
    BN = N // H
    prog, in_maps = _prepare(hidden_states, memory, W_mem, W_gate,
                             b_gate, norm_w)
    res = run_bass_kernel_spmd(prog, in_maps, core_ids=list(range(NC)))
    out = np.empty((B, N, D), dtype=np.float32)
    for c in range(NC):
        b, h = c // H, c % H
        out[b, h * BN:(h + 1) * BN, :] = res.results[c]["out"]
    return out


# revision 24
# speedup vs baseline: 2.0230x; 1.0439x over previous
"""MemoryGate kernel for Trainium2 (8 NeuronCores, SPMD), fp8 DoubleRow.

Math (per batch b):
    mp   = memory[b] @ W_mem.T                      [M, D]
    S    = hidden[b] @ mp.T / sqrt(D)               [N, M]
    A    = softmax(S, axis=-1)
    ctx  = A @ mp                                   [N, D]
    gate = sigmoid(hidden @ Wg_h.T + ctx @ Wg_c.T + b_gate)
    out  = rmsnorm(hidden + gate * ctx) * norm_w

Sharding: 8 cores = 4 batches x 2 N-halves. Each core computes mp for its
batch (duplicated across the pair) and processes N/2 = 2048 rows.

All big matmuls run fp8(e4m3) with perf_mode=DoubleRow: operands laid out
[128, ktiles, X] and sliced [:, 2t:2t+2, :] so each MM contracts K=256.
PSUM accumulates fp32. Softmax normalization is deferred: exp() writes
unnormalized fp8 attn weights (bias=-1 keeps the range within e4m3), the
fp32 row sums come from the activation accumulator, and 1/sum is applied
in the ctx PSUM drain (so ctx, ctxT and the gate see normalized values).

Layout strategy (per core, all [partition, free]):
    hT8     [D, BN] fp8 (host pre-transposed)  -> lhsT for scores / gate-G1
    memT8   [E, M], WmT8 [E, D] fp8            -> stage A operands
    mpT8    [D, M] fp8 (unscaled; 1/sqrt(D) folded into the exp scale)
    mp8     [M, D] fp8                          -> rhs for ctx
    scores PSUM [n-part, m-free]; exp -> attn8 fp8 (unnormalized)
    attn8 transposed on PE (fp8 128x128 blocks) -> attnT8 lhsT for ctx
    ctx drained with *1/rowsum -> ctx16 bf16; PE-transposed -> ctxT8 fp8
    gate rhs (Wgh/Wgc fp8) streamed from DRAM in [128,2,512] k-pair chunks
    b_gate added via a K=1 bf16 matmul into the same PSUM accumulation
    rmsnorm along free dim in [n-part, d-free] fp32 layout
"""

import math
import os
import sys

for _p in ("/opt/trn_rl_repo", "/root/.axon_site/_ro/trn_rl_repo"):
    if os.path.isdir(_p) and _p not in sys.path:
        sys.path.append(_p)

import numpy as np

P = 128


def build_program(BN=2048, M=2048, D=2048, E=1024, NB=512, FC=512,
                  stop_after=None):
    """Build the per-core Bass program. All shapes must divide evenly.

    stop_after: debug aid — truncate the program after a named phase
    ("A", "scores", "attnT", "ctx", "ctxT", "gate"); None = full kernel.
    """
    import concourse.tile as tile
    from concourse import bacc, mybir

    f32 = mybir.dt.float32
    bf16 = mybir.dt.bfloat16
    fp8 = mybir.dt.float8e4
    AF = mybir.ActivationFunctionType
    AX = mybir.AxisListType
    ALU = mybir.AluOpType
    DR = mybir.MatmulPerfMode.DoubleRow

    kE, kD, mT, nT = E // P, D // P, M // P, NB // P
    NBLK = BN // NB
    mFC, dFC = M // FC, D // FC
    kEh, kDh, mTh = kE // 2, kD // 2, mT // 2
    SCALE = 1.0 / math.sqrt(D)
    EXPB = -2.5          # exp(s/sqrt(D) - 2.5): keeps fp8 attn well under 240
                         # (empirical max s ~ 6.6; e^{6.6-2.5} ~ 60, Inf at 7.98)
    EPS = 1e-6

    nc = bacc.Bacc("TRN2", target_bir_lowering=False, debug=False)

    hT = nc.dram_tensor("hiddenT", [D, BN], fp8, kind="ExternalInput")
    hid = nc.dram_tensor("hidden", [BN, D], f32, kind="ExternalInput")
    memT = nc.dram_tensor("memT", [E, M], fp8, kind="ExternalInput")
    WmT = nc.dram_tensor("WmT", [E, D], fp8, kind="ExternalInput")
    WghT = nc.dram_tensor("WghT", [D, D], fp8, kind="ExternalInput")
    WgcT = nc.dram_tensor("WgcT", [D, D], fp8, kind="ExternalInput")
    bg = nc.dram_tensor("b_gate", [1, D], bf16, kind="ExternalInput")
    nw = nc.dram_tensor("norm_w", [1, D], f32, kind="ExternalInput")
    id8d = nc.dram_tensor("ident8", [P, P], fp8, kind="ExternalInput")
    id16d = nc.dram_tensor("ident16", [P, P], bf16, kind="ExternalInput")
    oned = nc.dram_tensor("ones", [1, P], bf16, kind="ExternalInput")
    out = nc.dram_tensor("out", [BN, D], f32, kind="ExternalOutput")

    with tile.TileContext(nc) as tc:
        with tc.tile_pool(name="const", bufs=1) as const:
            ident8 = const.tile([P, P], fp8, tag="id8", name="id8_sb")
            ident16 = const.tile([P, P], bf16, tag="id16", name="id16_sb")
            ones_sb = const.tile([1, P], bf16, tag="ones", name="ones_sb")
            bias_sb = const.tile([1, D], bf16, tag="bias", name="bias_sb")
            nw_sb = const.tile([P, D], f32, tag="nw", name="nw_sb")
            eps_t = const.tile([P, 1], f32, tag="eps", name="eps_sb")
            nc.vector.memset(eps_t, EPS)
            expb_t = const.tile([P, 1], f32, tag="expb", name="expb_sb")
            nc.vector.memset(expb_t, EXPB)

            # mpT8 + mp8 stay resident in SBUF for the whole kernel (8 MiB)
            hold_cm = tc.tile_pool(name="hold", bufs=1)
            hold = hold_cm.__enter__()
            mpT8 = hold.tile([P, kD, M], fp8, tag="mpT", name="mpT_sb")
            mp8 = hold.tile([P, mT, D], fp8, tag="mp", name="mp_sb")

            # ---------------- Stage A: mpT8 and mp8 (both unscaled) --------
            with (
                tc.tile_pool(name="a_in", bufs=1) as a_in,
                tc.tile_pool(name="a_ps", bufs=4, space="PSUM") as a_ps,
            ):
                # per-k-pair tiles so the first matmul starts after the
                # first pair lands instead of after all of E
                memT_p = [a_in.tile([P, 2, M], fp8, tag=f"memT{t}",
                                    name=f"memT_sb{t}") for t in range(kEh)]
                WmT_p = [a_in.tile([P, 2, D], fp8, tag=f"WmT{t}",
                                   name=f"WmT_sb{t}") for t in range(kEh)]
                for t in range(kEh):
                    for s in range(2):
                        k = 2 * t + s
                        nc.sync.dma_start(memT_p[t][:, s, :], memT[k * P:(k + 1) * P, :])
                        nc.sync.dma_start(WmT_p[t][:, s, :], WmT[k * P:(k + 1) * P, :])
                # consts are not needed until much later; issue their DMAs
                # behind the stage-A operands
                nc.sync.dma_start(ident8, id8d[:])
                nc.sync.dma_start(ident16, id16d[:])
                nc.sync.dma_start(ones_sb, oned[:])
                nc.sync.dma_start(bias_sb, bg[:])
                nc.gpsimd.dma_start(nw_sb, nw[:].partition_broadcast(P))
                # A1: mpT[d, m] = sum_e WmT[e, d] * memT[e, m]
                for dp in range(kD):
                    for mc in range(mFC):
                        ps = a_ps.tile([P, FC], f32, tag="ps", name=f"a1ps{dp}_{mc}")
                        for t in range(kEh):
                            nc.tensor.matmul(
                                ps,
                                WmT_p[t][:, :, dp * P:(dp + 1) * P],
                                memT_p[t][:, :, mc * FC:(mc + 1) * FC],
                                start=(t == 0), stop=(t == kEh - 1),
                                perf_mode=DR,
                            )
                        nc.scalar.copy(mpT8[:, dp, mc * FC:(mc + 1) * FC], ps)
                # A2: mp[m, d] = sum_e memT[e, m] * WmT[e, d]
                for mp_ in range(mT):
                    for dc in range(dFC):
                        ps = a_ps.tile([P, FC], f32, tag="ps", name=f"a2ps{mp_}_{dc}")
                        for t in range(kEh):
                            nc.tensor.matmul(
                                ps,
                                memT_p[t][:, :, mp_ * P:(mp_ + 1) * P],
                                WmT_p[t][:, :, dc * FC:(dc + 1) * FC],
                                start=(t == 0), stop=(t == kEh - 1),
                                perf_mode=DR,
                            )
                        nc.scalar.copy(mp8[:, mp_, dc * FC:(dc + 1) * FC], ps)

            # ---------------- Stage B: per N-block pipeline -----------------
            with (
                tc.tile_pool(name="b_big", bufs=1) as bb,
                tc.tile_pool(name="b_strm", bufs=8) as strm,
                tc.tile_pool(name="b_sm", bufs=2) as sm,
                tc.tile_pool(name="b_ps", bufs=6, space="PSUM") as bps,
            ):
                for blk in range(NBLK):
                    n0 = blk * NB
                    hT_sb = bb.tile([P, kD, NB], fp8, tag="hT", bufs=2,
                                    name=f"hT{blk}")
                    for k in range(kD):
                        nc.sync.dma_start(hT_sb[:, k, :], hT[k * P:(k + 1) * P, n0:n0 + NB])

                    if stop_after == "A":
                        if blk == 0:
                            for mt in range(mT):
                                cp = strm.tile([P, D], f32, tag="dbg", bufs=2,
                                               name=f"dbgA{mt}")
                                nc.vector.tensor_copy(cp, mp8[:, mt, :])
                                nc.sync.dma_start(out[mt * P:(mt + 1) * P, :], cp)
                        continue
                    # scores + exp (unnormalized attn) + row-chunk sums;
                    # attn->attnT transposes interleave into the mc loop so
                    # the PE always has independent queued work while the
                    # exp/cast drains of the previous chunk complete
                    attn = bb.tile([P, nT, M], bf16, tag="attn", name=f"attn{blk}")
                    attnT = bb.tile([P, mT, NB], fp8, tag="attnT", name=f"attnT{blk}")
                    sums = sm.tile([P, nT * mFC], f32, tag="sums", name=f"sums{blk}")
                    rs = sm.tile([P, nT], f32, tag="rs", name=f"rs{blk}")

                    def transp_attn(mt):
                        tp = bps.tile([P, NB], bf16, tag="tp", bufs=2,
                                      name=f"tpa{blk}_{mt}")
                        for i in range(nT):
                            nc.tensor.transpose(
                                tp[:, i * P:(i + 1) * P],
                                attn[:, i, mt * P:(mt + 1) * P], ident16)
                        nc.vector.tensor_copy(attnT[:, mt, :], tp)

                    for mc in range(mFC):
                        pss = [bps.tile([P, FC], f32, tag="ps", name=f"sc{blk}_{mc}_{i}")
                               for i in range(nT)]
                        for t in range(kDh):
                            for i in range(nT):
                                nc.tensor.matmul(
                                    pss[i],
                                    hT_sb[:, 2 * t:2 * t + 2, i * P:(i + 1) * P],
                                    mpT8[:, 2 * t:2 * t + 2, mc * FC:(mc + 1) * FC],
                                    start=(t == 0), stop=(t == kDh - 1),
                                    perf_mode=DR,
                                )
                        for i in range(nT):
                            nc.scalar.activation(
                                attn[:, i, mc * FC:(mc + 1) * FC], pss[i], AF.Exp,
                                scale=SCALE, bias=expb_t,
                                accum_out=sums[:, i * mFC + mc: i * mFC + mc + 1])
                        if mc >= 1:
                            for mt in range(4 * (mc - 1), 4 * mc):
                                transp_attn(mt)
                    # softmax denominators (normalization deferred to ctx drain)
                    for i in range(nT):
                        nc.vector.reduce_sum(
                            out=rs[:, i:i + 1], in_=sums[:, i * mFC:(i + 1) * mFC], axis=AX.X)
                    nc.vector.reciprocal(rs, rs)
                    for mt in range(4 * (mFC - 1), mT):
                        transp_attn(mt)

                    if stop_after == "scores":
                        for i in range(nT):
                            cp = strm.tile([P, M], f32, tag="dbg", bufs=2,
                                           name=f"dbgS{blk}_{i}")
                            nc.vector.tensor_copy(cp, attn[:, i, :])
                            nc.sync.dma_start(out[n0 + i * P:n0 + (i + 1) * P, :], cp)
                        continue

                    if stop_after == "attnT":
                        continue
                    # ctx = (attn @ mp) * 1/rowsum  (normalization in the drain)
                    ctxt = bb.tile([P, nT, D], bf16, tag="ctx", name=f"ctx{blk}")
                    for dc in range(dFC):
                        pss = [bps.tile([P, FC], f32, tag="ps", name=f"cx{blk}_{dc}_{i}")
                               for i in range(nT)]
                        for t in range(mTh):
                            for i in range(nT):
                                nc.tensor.matmul(
                                    pss[i],
                                    attnT[:, 2 * t:2 * t + 2, i * P:(i + 1) * P],
                                    mp8[:, 2 * t:2 * t + 2, dc * FC:(dc + 1) * FC],
                                    start=(t == 0), stop=(t == mTh - 1),
                                    perf_mode=DR,
                                )
                        for i in range(nT):
                            nc.scalar.mul(
                                ctxt[:, i, dc * FC:(dc + 1) * FC], pss[i], rs[:, i:i + 1])

                    if stop_after == "ctx":
                        for i in range(nT):
                            cp = strm.tile([P, D], f32, tag="dbg", bufs=2,
                                           name=f"dbgC{blk}_{i}")
                            nc.vector.tensor_copy(cp, ctxt[:, i, :])
                            nc.sync.dma_start(out[n0 + i * P:n0 + (i + 1) * P, :], cp)
                        continue
                    # gate = sigmoid(hidden @ WghT + ctx @ WgcT + b_gate)
                    # ctx->ctxT transposes run between G1(dc=0) and G2(dc=0):
                    # the G1 matmuls depend only on hT/Wgh, so the PE chews
                    # through them while the ctx PSUM drains finish
                    ctxT = bb.tile([P, kD, NB], fp8, tag="ctxT", name=f"ctxT{blk}")
                    gate = bb.tile([P, nT, D], bf16, tag="gate", name=f"gate{blk}")
                    for dc in range(dFC):
                        pss = [bps.tile([P, FC], f32, tag="ps", name=f"gt{blk}_{dc}_{i}")
                               for i in range(nT)]
                        for t in range(kDh):
                            ch = strm.tile([P, 2, FC], fp8, tag="rhs", name=f"g1ch{blk}_{dc}_{t}")
                            nc.sync.dma_start(
                                ch[:, 0, :], WghT[2 * t * P:(2 * t + 1) * P, dc * FC:(dc + 1) * FC])
                            nc.sync.dma_start(
                                ch[:, 1, :], WghT[(2 * t + 1) * P:(2 * t + 2) * P, dc * FC:(dc + 1) * FC])
                            for i in range(nT):
                                nc.tensor.matmul(
                                    pss[i],
                                    hT_sb[:, 2 * t:2 * t + 2, i * P:(i + 1) * P], ch,
                                    start=(t == 0), stop=False, perf_mode=DR)
                        if dc == 0:
                            for dt_ in range(kD):
                                tp = bps.tile([P, NB], bf16, tag="tp", bufs=2,
                                              name=f"tpc{blk}_{dt_}")
                                for i in range(nT):
                                    nc.tensor.transpose(
                                        tp[:, i * P:(i + 1) * P],
                                        ctxt[:, i, dt_ * P:(dt_ + 1) * P], ident16)
                                nc.vector.tensor_copy(ctxT[:, dt_, :], tp)
                        for t in range(kDh):
                            ch = strm.tile([P, 2, FC], fp8, tag="rhs", name=f"g2ch{blk}_{dc}_{t}")
                            nc.sync.dma_start(
                                ch[:, 0, :], WgcT[2 * t * P:(2 * t + 1) * P, dc * FC:(dc + 1) * FC])
                            nc.sync.dma_start(
                                ch[:, 1, :], WgcT[(2 * t + 1) * P:(2 * t + 2) * P, dc * FC:(dc + 1) * FC])
                            for i in range(nT):
                                nc.tensor.matmul(
                                    pss[i],
                                    ctxT[:, 2 * t:2 * t + 2, i * P:(i + 1) * P], ch,
                                    start=False, stop=False, perf_mode=DR)
                        for i in range(nT):
                            nc.tensor.matmul(
                                pss[i], ones_sb, bias_sb[:, dc * FC:(dc + 1) * FC],
                                start=False, stop=True)
                        for i in range(nT):
                            nc.scalar.activation(
                                gate[:, i, dc * FC:(dc + 1) * FC], pss[i], AF.Sigmoid)

                    if stop_after == "gate":
                        for i in range(nT):
                            cp = strm.tile([P, D], f32, tag="dbg", bufs=2,
                                           name=f"dbgG{blk}_{i}")
                            nc.vector.tensor_copy(cp, gate[:, i, :])
                            nc.sync.dma_start(out[n0 + i * P:n0 + (i + 1) * P, :], cp)
                        continue
                    # fused = hidden + gate*ctx; out = rmsnorm(fused) * norm_w
                    # chunked along D so it pipelines under the gate matmuls
                    for i in range(nT):
                        hid_t = strm.tile([P, D], f32, tag="hid", bufs=2, name=f"hid{blk}_{i}")
                        nc.sync.dma_start(hid_t, hid[n0 + i * P: n0 + (i + 1) * P, :])
                        fo = strm.tile([P, D], f32, tag="fo", bufs=2, name=f"fo{blk}_{i}")
                        sq = strm.tile([P, FC], f32, tag="sq", bufs=2, name=f"sq{blk}_{i}")
                        ssqp = sm.tile([P, dFC], f32, tag="ssq", name=f"ssqp{blk}_{i}")
                        for dc in range(dFC):
                            cs = slice(dc * FC, (dc + 1) * FC)
                            nc.vector.tensor_mul(fo[:, cs], gate[:, i, cs], ctxt[:, i, cs])
                            nc.vector.tensor_add(fo[:, cs], fo[:, cs], hid_t[:, cs])
                            nc.scalar.activation(sq, fo[:, cs], AF.Square,
                                                 accum_out=ssqp[:, dc:dc + 1])
                        ssq = sm.tile([P, 1], f32, tag="ssq", name=f"ssq{blk}_{i}")
                        nc.vector.reduce_sum(out=ssq, in_=ssqp, axis=AX.X)
                        rstd = sm.tile([P, 1], f32, tag="rstd", name=f"rstd{blk}_{i}")
                        nc.scalar.activation(rstd, ssq, AF.Sqrt, bias=eps_t, scale=1.0 / D)
                        nc.vector.reciprocal(rstd, rstd)
                        for dc in range(dFC):
                            cs = slice(dc * FC, (dc + 1) * FC)
                            # out = (fo * rstd) * norm_w in one DVE pass
                            nc.vector.scalar_tensor_tensor(
                                fo[:, cs], fo[:, cs], rstd, nw_sb[:, cs],
                                op0=ALU.mult, op1=ALU.mult)
                            nc.sync.dma_start(out[n0 + i * P: n0 + (i + 1) * P, cs], fo[:, cs])

            hold_cm.__exit__(None, None, None)

    nc.compile()
    return nc


_PROG_CACHE = {}


def _get_program(key, **kw):
    if key not in _PROG_CACHE:
        _PROG_CACHE[key] = build_program(**kw)
    return _PROG_CACHE[key]


def prepare(inputs):
    """Build (prog, in_maps) for the 8-core SPMD run."""
    return _prepare(inputs["hidden_states"], inputs["memory"], inputs["W_mem"],
                    inputs["W_gate"], inputs["b_gate"], inputs["norm_w"])


def _prepare(hidden_states, memory, W_mem, W_gate, b_gate, norm_w):
    B, N, D = hidden_states.shape
    _, M, E = memory.shape
    NC = 8
    H = NC // B                      # N-splits per batch (2)
    BN = N // H                      # rows per core (2048)

    prog = _get_program(("fp8", BN, M, D, E), BN=BN, M=M, D=D, E=E)

    import ml_dtypes
    f32 = np.float32
    bf16 = ml_dtypes.bfloat16
    fp8 = ml_dtypes.float8_e4m3
    WmT = np.ascontiguousarray(W_mem.T).astype(fp8)
    WghT = np.ascontiguousarray(W_gate[:, :D].T).astype(fp8)
    WgcT = np.ascontiguousarray(W_gate[:, D:].T).astype(fp8)
    bg = np.ascontiguousarray(b_gate[None, :]).astype(bf16)
    nw = np.ascontiguousarray(norm_w[None, :], dtype=f32)
    ident8 = np.eye(P, dtype=f32).astype(fp8)
    ident16 = np.eye(P, dtype=f32).astype(bf16)
    ones = np.ones((1, P), dtype=bf16)

    in_maps = []
    for c in range(NC):
        b, h = c // H, c % H
        hs = hidden_states[b, h * BN:(h + 1) * BN, :]
        in_maps.append({
            "hiddenT": np.ascontiguousarray(hs.T).astype(fp8),
            "hidden": np.ascontiguousarray(hs, dtype=f32),
            "memT": np.ascontiguousarray(memory[b].T).astype(fp8),
            "WmT": WmT, "WghT": WghT, "WgcT": WgcT,
            "b_gate": bg, "norm_w": nw,
            "ident8": ident8, "ident16": ident16, "ones": ones,
        })
    return prog, in_maps


def kernel(hidden_states, memory, W_mem, W_gate, b_gate, norm_w):
    from concourse.bass_utils import run_bass_kernel_spmd

    B, N, D = hidden_states.shape
    NC = 8
    H = NC // B
    BN = N // H
    prog, in_maps = _prepare(hidden_states, memory, W_mem, W_gate,
                             b_gate, norm_w)
    res = run_bass_kernel_spmd(prog, in_maps, core_ids=list(range(NC)))
    out = np.empty((B, N, D), dtype=np.float32)
    for c in range(NC):
        b, h = c // H, c % H
        out[b, h * BN:(h + 1) * BN, :] = res.results[c]["out"]
    return out
